# revision 14
# baseline (speedup 1.0000x reference)
"""Trainium2 Bass kernel for nn_BitwiseCellularAutomata.

The reference runs a 100-step cellular automaton:
    h0 = bit_stimuli.int8         [B, N]
    W0 = identity                 [B, N, N]
    step: E = bit3(h); P = bit2(h)
          ce    = (W @ E > 0)
          E_new = E ^ ce
          P_new = P & ~E_new
          W    |= outer(E_new, ~P_new)
          h     = E_new<<3 | P_new<<2 | (h & 3)

Because W0 is the identity, at the very first step ce == E, so
E_new == E ^ E == 0 for every cell.  Then P_new == P, the wiring
outer-product is identically zero (E_new == 0), and W stays the
identity.  From step 2 on the state is a fixed point (E stays 0).
Hence for ANY input values and ANY steps >= 1:
    h_final = bit_stimuli & 7   (int8)     [bit 3 cleared, bits 2..0 kept]
    W_final = identity          (int8)
and for steps == 0: h = bit_stimuli.astype(int8), W = identity.
(Verified bit-exact against the reference scan output.)

Sharding: pure data parallel, batch 16 -> 2 per core across 8 cores,
no cross-device communication.

The runtime contract of run_bass_kernel_spmd zero-initializes
ExternalOutput buffers (native path: out_map = np.zeros handed to
run_neff; axon/PJRT path: zero buffers are passed as operands bound to
the output tensors and donated).  Kernels that don't write every
element rely on that, so W only needs its diagonal written: one
single-byte-descriptor scatter DMA per batch (2048 descriptors each,
at the cost model's per-descriptor floor), sourced from a [128, 16]
ones tile built by a single memset — no index generation needed at all.

Raw Bass (no TileContext): the Tile tail drain waits on one semaphore
per engine/DMA lane it saw, which exceeds this toolchain's
per-instruction sync-wait slot limit; with manual semaphores every
instruction carries at most one wait, and we also skip Tile's
tail barrier overhead.

Ring layout (cost model 6183 ns/core; the critical path is the
irreducible x-in -> DVE -> h-out fixed-latency chain, under which the
W scatter fully hides):
  * sync (qSP HWDGE): x-in, then h-out after DVE signals, then final
    completion waits — the h chain owns this ring end to end.
  * scalar (qAct HWDGE): both W diagonal scatters.
  * vector: memset of the ones tile (releases W immediately), then
    h = low_byte(x) & 7 from a byte view of the int32 input
    ((x & 0xFF) & 7 == x & 7, no dtype cast).
"""

import numpy as np

B, N = 16, 2048
N_CORES = 8
B_SHARD = B // N_CORES  # 2 batches per core
P = 128
RB = N // P  # 16 diagonal blocks per batch

_BUILD_CACHE = {}


def _strip_dead_preamble(nc):
    """Remove Bass-emitted boilerplate this kernel provably never needs:

    * the four const-tile memsets (walrus itself warns they have no reader);
      they run serially on the Pool engine and gate the preamble barrier;
    * the per-engine zero/broadcast register initializations (a BIR scan
      shows zero instructions read any register); SP's moves delayed its
      first DMA issue by ~250 ns;
    * the preamble and exit all-engine barriers (drains + event-semaphore
      butterfly).  The drains carry no semaphore-reset duty
      (is_reset_sema=None); semaphore zeroing is runtime-level, which is
      also why repeated executions stay correct.  All user-visible ordering
      runs through explicit semaphores, and SP's final wait_ge on the
      output-DMA completion semaphores is the completion guarantee, so the
      barriers are pure ceremony here.  Verified by repeated-execution
      hardware soaks (bit-exact).

    Cost-model gain: ~1.2 us total (6183 -> 4968).

    Every removal is independent and individually safe, so the pass is
    best-effort: if the surrounding toolchain ever changes shape, a failed
    removal is skipped and the kernel still builds (just without that
    optimization)."""
    try:
        blocks = list(nc.m.functions[0].blocks)
    except Exception:
        return nc
    for bi, blk in enumerate(blocks):
        try:
            il = blk.instructions
            barrier_block = bi == 0 or blk.name.endswith("_end")
        except Exception:
            continue
        for inst in list(il):
            try:
                tn = type(inst).__name__
                if tn == "InstMemset" and inst.outs and "const-" in str(inst.outs[0]):
                    il.remove(inst)
                elif tn == "InstRegisterMove" and inst.outs:
                    s = str(inst.outs[0])
                    if any(
                        p in s for p in ("PE_", "Pool_", "SP_", "DVE_", "Activation_")
                    ):
                        il.remove(inst)
                elif barrier_block and tn in ("InstDrain", "InstEventSemaphore"):
                    il.remove(inst)
            except Exception:
                continue
    return nc


def _build_bass(mask):
    """Build the per-core Bass module. mask=None means plain int8 cast."""
    import concourse.bass as bass
    import concourse.mybir as mybir

    nc = bass.Bass()
    x = nc.declare_dram_parameter(
        "bit_stimuli", [B_SHARD, N], mybir.dt.int32, isOutput=False
    )
    h_out = nc.declare_dram_parameter(
        "h_out", [B_SHARD, N], mybir.dt.int8, isOutput=True
    )
    w_out = nc.declare_dram_parameter(
        "W_out", [B_SHARD, N, N], mybir.dt.int8, isOutput=True
    )
    FB = B_SHARD * N * 4 // P  # bytes per partition of the raw int32 input
    w_handle = w_out[:].tensor

    with (
        nc.sbuf_tensor([P, FB], mybir.dt.int8) as xb,
        nc.sbuf_tensor([P, FB // 4], mybir.dt.int8) as ht,
        nc.sbuf_tensor([P, RB], mybir.dt.int8) as T16,
        nc.semaphore() as s_in,
        nc.semaphore() as s_T,
        nc.semaphore() as s_h,
        nc.semaphore() as s_out,
        nc.semaphore() as s_ho,
        nc.Block() as block,
    ):
        x_bytes = x[:].bitcast(mybir.dt.int8).flatten().rearrange("(p f) -> p f", p=P)
        h_dest = h_out[:].flatten().rearrange("(p f) -> p f", p=P)

        @block.vector
        def _(vector):
            vector.memset(T16[:], 1).then_inc(s_T, 1)
            vector.wait_ge(s_in, 16)
            # low byte of each little-endian int32; (x & 0xFF) & 7 == x & 7
            x_lo = xb[:].rearrange("p (f four) -> p f four", four=4)[:, :, 0]
            if mask is None:
                vector.tensor_copy(ht[:], x_lo).then_inc(s_h, 1)
            else:
                vector.tensor_scalar(
                    ht[:], x_lo, mask, None, mybir.AluOpType.bitwise_and
                ).then_inc(s_h, 1)

        @block.scalar
        def _(scalar):
            scalar.wait_ge(s_T, 1)
            for b in range(B_SHARD):
                # element (p, r) -> W[b, 128r + p, 128r + p] = 1
                src = bass.AP(T16[:].tensor, 0, [[RB, P], [1, RB], [1, 1]])
                dest = bass.AP(
                    w_handle, b * N * N, [[N + 1, P], [P * N + P, RB], [1, 1]]
                )
                with nc.allow_non_contiguous_dma(
                    reason="single-byte diagonal scatter is intended"
                ):
                    scalar.dma_start(out=dest, in_=src).then_inc(s_out, 16)

        @block.sync
        def _(sync):
            sync.dma_start(out=xb[:], in_=x_bytes).then_inc(s_in, 16)
            sync.wait_ge(s_h, 1)
            sync.dma_start(out=h_dest, in_=ht[:]).then_inc(s_ho, 16)
            sync.wait_ge(s_out, 32)
            sync.wait_ge(s_ho, 16)

    return _strip_dead_preamble(nc)


def _get_nc(mask):
    if mask not in _BUILD_CACHE:
        _BUILD_CACHE[mask] = _build_bass(mask)
    return _BUILD_CACHE[mask]


def run_sharded(bit_stimuli, steps, trace=False):
    """Shard, run on 8 cores, gather. Returns ((h, W), BassKernelResults)."""
    from concourse.bass_utils import run_bass_kernel_spmd

    bs = np.ascontiguousarray(np.asarray(bit_stimuli, dtype=np.int32))
    assert bs.shape == (B, N), bs.shape
    steps = int(np.asarray(steps))
    mask = None if steps == 0 else 7

    nc = _get_nc(mask)
    in_maps = [
        {"bit_stimuli": bs[i * B_SHARD : (i + 1) * B_SHARD]} for i in range(N_CORES)
    ]
    res = run_bass_kernel_spmd(nc, in_maps, core_ids=list(range(N_CORES)), trace=trace)

    h = np.concatenate(
        [np.asarray(r["h_out"], dtype=np.int8) for r in res.results], axis=0
    )
    W = np.concatenate(
        [np.asarray(r["W_out"], dtype=np.int8) for r in res.results], axis=0
    )
    return (h, W), res


def kernel(**inputs):
    (h, W), _ = run_sharded(inputs["bit_stimuli"], inputs["steps"])
    return h, W


# revision 15
# speedup vs baseline: 1.0102x; 1.0102x over previous
"""Trainium2 Bass kernel for nn_BitwiseCellularAutomata.

The reference runs a 100-step cellular automaton:
    h0 = bit_stimuli.int8         [B, N]
    W0 = identity                 [B, N, N]
    step: E = bit3(h); P = bit2(h)
          ce    = (W @ E > 0)
          E_new = E ^ ce
          P_new = P & ~E_new
          W    |= outer(E_new, ~P_new)
          h     = E_new<<3 | P_new<<2 | (h & 3)

Because W0 is the identity, at the very first step ce == E, so
E_new == E ^ E == 0 for every cell.  Then P_new == P, the wiring
outer-product is identically zero (E_new == 0), and W stays the
identity.  From step 2 on the state is a fixed point (E stays 0).
Hence for ANY input values and ANY steps >= 1:
    h_final = bit_stimuli & 7   (int8)     [bit 3 cleared, bits 2..0 kept]
    W_final = identity          (int8)
and for steps == 0: h = bit_stimuli.astype(int8), W = identity.
(Verified bit-exact against the reference scan output.)

Sharding: pure data parallel, batch 16 -> 2 per core across 8 cores,
no cross-device communication.

The runtime contract of run_bass_kernel_spmd zero-initializes
ExternalOutput buffers (native path: out_map = np.zeros handed to
run_neff; axon/PJRT path: zero buffers are passed as operands bound to
the output tensors and donated).  Kernels that don't write every
element rely on that, so W only needs its diagonal written: one
single-byte-descriptor scatter DMA per batch (2048 descriptors each,
at the cost model's per-descriptor floor), sourced from a [128, 16]
ones tile built by a single memset — no index generation needed at all.

Raw Bass (no TileContext): the Tile tail drain waits on one semaphore
per engine/DMA lane it saw, which exceeds this toolchain's
per-instruction sync-wait slot limit; with manual semaphores every
instruction carries at most one wait, and we also skip Tile's
tail barrier overhead.

Ring layout (cost model 6183 ns/core; the critical path is the
irreducible x-in -> DVE -> h-out fixed-latency chain, under which the
W scatter fully hides):
  * sync (qSP HWDGE): x-in, then h-out after DVE signals, then final
    completion waits — the h chain owns this ring end to end.
  * scalar (qAct HWDGE): both W diagonal scatters.
  * vector: memset of the ones tile (releases W immediately), then
    h = low_byte(x) & 7 from a byte view of the int32 input
    ((x & 0xFF) & 7 == x & 7, no dtype cast).
"""

import numpy as np

B, N = 16, 2048
N_CORES = 8
B_SHARD = B // N_CORES  # 2 batches per core
P = 128
RB = N // P  # 16 diagonal blocks per batch

_BUILD_CACHE = {}


def _strip_dead_preamble(nc):
    """Remove Bass-emitted boilerplate this kernel provably never needs:

    * the four const-tile memsets (walrus itself warns they have no reader);
      they run serially on the Pool engine and gate the preamble barrier;
    * the per-engine zero/broadcast register initializations (a BIR scan
      shows zero instructions read any register); SP's moves delayed its
      first DMA issue by ~250 ns;
    * the preamble and exit all-engine barriers (drains + event-semaphore
      butterfly).  The drains carry no semaphore-reset duty
      (is_reset_sema=None); semaphore zeroing is runtime-level, which is
      also why repeated executions stay correct.  All user-visible ordering
      runs through explicit semaphores, and SP's final wait_ge on the
      output-DMA completion semaphores is the completion guarantee, so the
      barriers are pure ceremony here.  Verified by repeated-execution
      hardware soaks (bit-exact).

    Cost-model gain: ~1.2 us total (6183 -> 4968).

    Every removal is independent and individually safe, so the pass is
    best-effort: if the surrounding toolchain ever changes shape, a failed
    removal is skipped and the kernel still builds (just without that
    optimization)."""
    try:
        blocks = list(nc.m.functions[0].blocks)
    except Exception:
        return nc
    for bi, blk in enumerate(blocks):
        try:
            il = blk.instructions
            barrier_block = bi == 0 or blk.name.endswith("_end")
        except Exception:
            continue
        for inst in list(il):
            try:
                tn = type(inst).__name__
                if tn == "InstMemset" and inst.outs and "const-" in str(inst.outs[0]):
                    il.remove(inst)
                elif tn == "InstRegisterMove" and inst.outs:
                    s = str(inst.outs[0])
                    if any(
                        p in s for p in ("PE_", "Pool_", "SP_", "DVE_", "Activation_")
                    ):
                        il.remove(inst)
                elif barrier_block and tn in ("InstDrain", "InstEventSemaphore"):
                    il.remove(inst)
            except Exception:
                continue
    # Hoist SP's x-in DMA into the entry block ahead of SP's branch so it
    # issues at t=0 instead of after the 50 ns branch (engine instructions
    # in the entry block are structurally normal — the stripped barrier
    # drains lived there).  Best-effort like the rest of this pass.
    try:
        b0 = blocks[0]
        spb = next(b for b in blocks if "_SP_" in b.name)
        il = spb.instructions
        dma = next(i for i in il if type(i).__name__ == "InstDMACopy")
        il.remove(dma)
        il0 = b0.instructions
        idx = next(
            i
            for i, inst in enumerate(list(il0))
            if type(inst).__name__ == "InstUnconditionalBranch"
            and str(inst.engine).endswith("SP")
        )
        il0.insert(idx, dma)
    except Exception:
        pass
    return nc


def _build_bass(mask):
    """Build the per-core Bass module. mask=None means plain int8 cast."""
    import concourse.bass as bass
    import concourse.mybir as mybir

    nc = bass.Bass()
    x = nc.declare_dram_parameter(
        "bit_stimuli", [B_SHARD, N], mybir.dt.int32, isOutput=False
    )
    h_out = nc.declare_dram_parameter(
        "h_out", [B_SHARD, N], mybir.dt.int8, isOutput=True
    )
    w_out = nc.declare_dram_parameter(
        "W_out", [B_SHARD, N, N], mybir.dt.int8, isOutput=True
    )
    FB = B_SHARD * N * 4 // P  # bytes per partition of the raw int32 input
    w_handle = w_out[:].tensor

    with (
        nc.sbuf_tensor([P, FB], mybir.dt.int8) as xb,
        nc.sbuf_tensor([P, FB // 4], mybir.dt.int8) as ht,
        nc.sbuf_tensor([P, RB], mybir.dt.int8) as T16,
        nc.semaphore() as s_in,
        nc.semaphore() as s_T,
        nc.semaphore() as s_h,
        nc.semaphore() as s_out,
        nc.semaphore() as s_ho,
        nc.Block() as block,
    ):
        x_bytes = x[:].bitcast(mybir.dt.int8).flatten().rearrange("(p f) -> p f", p=P)
        h_dest = h_out[:].flatten().rearrange("(p f) -> p f", p=P)

        @block.vector
        def _(vector):
            vector.memset(T16[:], 1).then_inc(s_T, 1)
            vector.wait_ge(s_in, 16)
            # low byte of each little-endian int32; (x & 0xFF) & 7 == x & 7
            x_lo = xb[:].rearrange("p (f four) -> p f four", four=4)[:, :, 0]
            if mask is None:
                vector.tensor_copy(ht[:], x_lo).then_inc(s_h, 1)
            else:
                vector.tensor_scalar(
                    ht[:], x_lo, mask, None, mybir.AluOpType.bitwise_and
                ).then_inc(s_h, 1)

        @block.scalar
        def _(scalar):
            scalar.wait_ge(s_T, 1)
            for b in range(B_SHARD):
                # element (p, r) -> W[b, 128r + p, 128r + p] = 1
                src = bass.AP(T16[:].tensor, 0, [[RB, P], [1, RB], [1, 1]])
                dest = bass.AP(
                    w_handle, b * N * N, [[N + 1, P], [P * N + P, RB], [1, 1]]
                )
                with nc.allow_non_contiguous_dma(
                    reason="single-byte diagonal scatter is intended"
                ):
                    scalar.dma_start(out=dest, in_=src).then_inc(s_out, 16)

        @block.sync
        def _(sync):
            sync.dma_start(out=xb[:], in_=x_bytes).then_inc(s_in, 16)
            sync.wait_ge(s_h, 1)
            sync.dma_start(out=h_dest, in_=ht[:]).then_inc(s_ho, 16)
            sync.wait_ge(s_out, 32)
            sync.wait_ge(s_ho, 16)

    return _strip_dead_preamble(nc)


def _get_nc(mask):
    if mask not in _BUILD_CACHE:
        _BUILD_CACHE[mask] = _build_bass(mask)
    return _BUILD_CACHE[mask]


def run_sharded(bit_stimuli, steps, trace=False):
    """Shard, run on 8 cores, gather. Returns ((h, W), BassKernelResults)."""
    from concourse.bass_utils import run_bass_kernel_spmd

    bs = np.ascontiguousarray(np.asarray(bit_stimuli, dtype=np.int32))
    assert bs.shape == (B, N), bs.shape
    steps = int(np.asarray(steps))
    mask = None if steps == 0 else 7

    nc = _get_nc(mask)
    in_maps = [
        {"bit_stimuli": bs[i * B_SHARD : (i + 1) * B_SHARD]} for i in range(N_CORES)
    ]
    res = run_bass_kernel_spmd(nc, in_maps, core_ids=list(range(N_CORES)), trace=trace)

    h = np.concatenate(
        [np.asarray(r["h_out"], dtype=np.int8) for r in res.results], axis=0
    )
    W = np.concatenate(
        [np.asarray(r["W_out"], dtype=np.int8) for r in res.results], axis=0
    )
    return (h, W), res


def kernel(**inputs):
    (h, W), _ = run_sharded(inputs["bit_stimuli"], inputs["steps"])
    return h, W


# revision 16
# speedup vs baseline: 1.0205x; 1.0103x over previous
"""Trainium2 Bass kernel for nn_BitwiseCellularAutomata.

The reference runs a 100-step cellular automaton:
    h0 = bit_stimuli.int8         [B, N]
    W0 = identity                 [B, N, N]
    step: E = bit3(h); P = bit2(h)
          ce    = (W @ E > 0)
          E_new = E ^ ce
          P_new = P & ~E_new
          W    |= outer(E_new, ~P_new)
          h     = E_new<<3 | P_new<<2 | (h & 3)

Because W0 is the identity, at the very first step ce == E, so
E_new == E ^ E == 0 for every cell.  Then P_new == P, the wiring
outer-product is identically zero (E_new == 0), and W stays the
identity.  From step 2 on the state is a fixed point (E stays 0).
Hence for ANY input values and ANY steps >= 1:
    h_final = bit_stimuli & 7   (int8)     [bit 3 cleared, bits 2..0 kept]
    W_final = identity          (int8)
and for steps == 0: h = bit_stimuli.astype(int8), W = identity.
(Verified bit-exact against the reference scan output.)

Sharding: pure data parallel, batch 16 -> 2 per core across 8 cores,
no cross-device communication.

The runtime contract of run_bass_kernel_spmd zero-initializes
ExternalOutput buffers (native path: out_map = np.zeros handed to
run_neff; axon/PJRT path: zero buffers are passed as operands bound to
the output tensors and donated).  Kernels that don't write every
element rely on that, so W only needs its diagonal written: one
single-byte-descriptor scatter DMA per batch (2048 descriptors each,
at the cost model's per-descriptor floor), sourced from a [128, 16]
ones tile built by a single memset — no index generation needed at all.

Raw Bass (no TileContext): the Tile tail drain waits on one semaphore
per engine/DMA lane it saw, which exceeds this toolchain's
per-instruction sync-wait slot limit; with manual semaphores every
instruction carries at most one wait, and we also skip Tile's
tail barrier overhead.

Ring layout (cost model 6183 ns/core; the critical path is the
irreducible x-in -> DVE -> h-out fixed-latency chain, under which the
W scatter fully hides):
  * sync (qSP HWDGE): x-in, then h-out after DVE signals, then final
    completion waits — the h chain owns this ring end to end.
  * scalar (qAct HWDGE): both W diagonal scatters.
  * vector: memset of the ones tile (releases W immediately), then
    h = low_byte(x) & 7 from a byte view of the int32 input
    ((x & 0xFF) & 7 == x & 7, no dtype cast).
"""

import numpy as np

B, N = 16, 2048
N_CORES = 8
B_SHARD = B // N_CORES  # 2 batches per core
P = 128
RB = N // P  # 16 diagonal blocks per batch

_BUILD_CACHE = {}


def _strip_dead_preamble(nc):
    """Remove Bass-emitted boilerplate this kernel provably never needs:

    * the four const-tile memsets (walrus itself warns they have no reader);
      they run serially on the Pool engine and gate the preamble barrier;
    * the per-engine zero/broadcast register initializations (a BIR scan
      shows zero instructions read any register); SP's moves delayed its
      first DMA issue by ~250 ns;
    * the preamble and exit all-engine barriers (drains + event-semaphore
      butterfly).  The drains carry no semaphore-reset duty
      (is_reset_sema=None); semaphore zeroing is runtime-level, which is
      also why repeated executions stay correct.  All user-visible ordering
      runs through explicit semaphores, and SP's final wait_ge on the
      output-DMA completion semaphores is the completion guarantee, so the
      barriers are pure ceremony here.  Verified by repeated-execution
      hardware soaks (bit-exact).

    Cost-model gain: ~1.2 us total (6183 -> 4968).

    Every removal is independent and individually safe, so the pass is
    best-effort: if the surrounding toolchain ever changes shape, a failed
    removal is skipped and the kernel still builds (just without that
    optimization)."""
    try:
        blocks = list(nc.m.functions[0].blocks)
    except Exception:
        return nc
    for bi, blk in enumerate(blocks):
        try:
            il = blk.instructions
            barrier_block = bi == 0 or blk.name.endswith("_end")
        except Exception:
            continue
        for inst in list(il):
            try:
                tn = type(inst).__name__
                if tn == "InstMemset" and inst.outs and "const-" in str(inst.outs[0]):
                    il.remove(inst)
                elif tn == "InstRegisterMove" and inst.outs:
                    s = str(inst.outs[0])
                    if any(
                        p in s for p in ("PE_", "Pool_", "SP_", "DVE_", "Activation_")
                    ):
                        il.remove(inst)
                elif barrier_block and tn in ("InstDrain", "InstEventSemaphore"):
                    il.remove(inst)
            except Exception:
                continue
    # Drop the terminal branches into the (emptied) exit block: each
    # engine's stream then simply ends at its last real instruction,
    # saving the 50 ns branch decode after SP's final wait.
    try:
        for blk in blocks[1:]:
            il = blk.instructions
            for inst in list(il):
                if type(inst).__name__ == "InstUnconditionalBranch":
                    il.remove(inst)
    except Exception:
        pass
    # Hoist SP's x-in DMA into the entry block ahead of SP's branch so it
    # issues at t=0 instead of after the 50 ns branch (engine instructions
    # in the entry block are structurally normal — the stripped barrier
    # drains lived there).  Best-effort like the rest of this pass.
    try:
        b0 = blocks[0]
        spb = next(b for b in blocks if "_SP_" in b.name)
        il = spb.instructions
        dma = next(i for i in il if type(i).__name__ == "InstDMACopy")
        il.remove(dma)
        il0 = b0.instructions
        idx = next(
            i
            for i, inst in enumerate(list(il0))
            if type(inst).__name__ == "InstUnconditionalBranch"
            and str(inst.engine).endswith("SP")
        )
        il0.insert(idx, dma)
    except Exception:
        pass
    return nc


def _build_bass(mask):
    """Build the per-core Bass module. mask=None means plain int8 cast."""
    import concourse.bass as bass
    import concourse.mybir as mybir

    nc = bass.Bass()
    x = nc.declare_dram_parameter(
        "bit_stimuli", [B_SHARD, N], mybir.dt.int32, isOutput=False
    )
    h_out = nc.declare_dram_parameter(
        "h_out", [B_SHARD, N], mybir.dt.int8, isOutput=True
    )
    w_out = nc.declare_dram_parameter(
        "W_out", [B_SHARD, N, N], mybir.dt.int8, isOutput=True
    )
    FB = B_SHARD * N * 4 // P  # bytes per partition of the raw int32 input
    w_handle = w_out[:].tensor

    with (
        nc.sbuf_tensor([P, FB], mybir.dt.int8) as xb,
        nc.sbuf_tensor([P, FB // 4], mybir.dt.int8) as ht,
        nc.sbuf_tensor([P, RB], mybir.dt.int8) as T16,
        nc.semaphore() as s_in,
        nc.semaphore() as s_T,
        nc.semaphore() as s_h,
        nc.semaphore() as s_out,
        nc.semaphore() as s_ho,
        nc.Block() as block,
    ):
        x_bytes = x[:].bitcast(mybir.dt.int8).flatten().rearrange("(p f) -> p f", p=P)
        h_dest = h_out[:].flatten().rearrange("(p f) -> p f", p=P)

        @block.vector
        def _(vector):
            vector.memset(T16[:], 1).then_inc(s_T, 1)
            vector.wait_ge(s_in, 16)
            # low byte of each little-endian int32; (x & 0xFF) & 7 == x & 7
            x_lo = xb[:].rearrange("p (f four) -> p f four", four=4)[:, :, 0]
            if mask is None:
                vector.tensor_copy(ht[:], x_lo).then_inc(s_h, 1)
            else:
                vector.tensor_scalar(
                    ht[:], x_lo, mask, None, mybir.AluOpType.bitwise_and
                ).then_inc(s_h, 1)

        @block.scalar
        def _(scalar):
            scalar.wait_ge(s_T, 1)
            for b in range(B_SHARD):
                # element (p, r) -> W[b, 128r + p, 128r + p] = 1
                src = bass.AP(T16[:].tensor, 0, [[RB, P], [1, RB], [1, 1]])
                dest = bass.AP(
                    w_handle, b * N * N, [[N + 1, P], [P * N + P, RB], [1, 1]]
                )
                with nc.allow_non_contiguous_dma(
                    reason="single-byte diagonal scatter is intended"
                ):
                    scalar.dma_start(out=dest, in_=src).then_inc(s_out, 16)

        @block.sync
        def _(sync):
            sync.dma_start(out=xb[:], in_=x_bytes).then_inc(s_in, 16)
            sync.wait_ge(s_h, 1)
            sync.dma_start(out=h_dest, in_=ht[:]).then_inc(s_ho, 16)
            sync.wait_ge(s_out, 32)
            sync.wait_ge(s_ho, 16)

    return _strip_dead_preamble(nc)


def _get_nc(mask):
    if mask not in _BUILD_CACHE:
        _BUILD_CACHE[mask] = _build_bass(mask)
    return _BUILD_CACHE[mask]


def run_sharded(bit_stimuli, steps, trace=False):
    """Shard, run on 8 cores, gather. Returns ((h, W), BassKernelResults)."""
    from concourse.bass_utils import run_bass_kernel_spmd

    bs = np.ascontiguousarray(np.asarray(bit_stimuli, dtype=np.int32))
    assert bs.shape == (B, N), bs.shape
    steps = int(np.asarray(steps))
    mask = None if steps == 0 else 7

    nc = _get_nc(mask)
    in_maps = [
        {"bit_stimuli": bs[i * B_SHARD : (i + 1) * B_SHARD]} for i in range(N_CORES)
    ]
    res = run_bass_kernel_spmd(nc, in_maps, core_ids=list(range(N_CORES)), trace=trace)

    h = np.concatenate(
        [np.asarray(r["h_out"], dtype=np.int8) for r in res.results], axis=0
    )
    W = np.concatenate(
        [np.asarray(r["W_out"], dtype=np.int8) for r in res.results], axis=0
    )
    return (h, W), res


def kernel(**inputs):
    (h, W), _ = run_sharded(inputs["bit_stimuli"], inputs["steps"])
    return h, W
